# revision 15
# baseline (speedup 1.0000x reference)
"""DeepSeek-V2 MoE MLP kernel for Trainium2, 8 NeuronCores, expert-parallel.

Sharding: 2 routed experts per core; shared-expert intermediate dim sharded
8 ways (352 cols/core); router replicated. Each core produces a partial
[2048, 2048] output (its experts' routed contributions + its shared slice);
the full output is the sum of the 8 partials.

v3: bf16 expert/shared matmuls (fp32 router for exact top-4 selection),
xT resident in SBUF (no DRAM bounce), host-side transpose of x, per-core
router-column permutation (local experts first), CAP 768->640, dispatch
math emitted between the two shared-expert halves so the prefix-scan
latency hides under matmuls, explicit engine assignment (PSUM->SBUF copies
on vector, Silu/Exp on scalar), gpsimd queue ordered to avoid head-of-line
blocking of the output writes.

Problem shapes (hardcoded per contract):
  T=2048 tokens, D=2048 hidden, I=1408 moe inter, E=16 experts, K=4,
  SHARED_INTER=2816. Reference capacity is 768; we use CAP=640: per-expert
  counts concentrate near T*K/E=512 +- ~22, so 640 keeps >4 sigma of margin
  and neither path ever actually drops a token.
"""

import numpy as np

T = 2048
D = 2048
MOE_I = 1408
E = 16
TOPK = 4
SI = 2816
SI_LOC = SI // 8  # 352
CAP = 640
NCORES = 8
EXP_PER_CORE = 2
NT = T // 128     # 16 token tiles
ND = D // 128     # 16 d tiles
NI = MOE_I // 128  # 11 inter tiles
NS = CAP // 128   # 5 slot tiles per expert
# OOB marker for unfilled table rows. Must exceed every bounds_check but stay
# small enough that sentinel * row_elems fits in int32 (the DGE index math).
SENTINEL = 1.0e5


def build_nc():
    """Build the Bass/Tile program (identical for all cores)."""
    import concourse.bacc as bacc
    import concourse.bass as bass
    import concourse.mybir as mybir
    import concourse.tile as tile

    dt = mybir.dt
    f32 = dt.float32
    bf16 = dt.bfloat16
    i32 = dt.int32
    AF = mybir.ActivationFunctionType
    ALU = mybir.AluOpType

    nc = bacc.Bacc("TRN2", target_bir_lowering=False, debug=False,
                   num_devices=NCORES)

    # ---------------- I/O ----------------
    # xt32: x transposed and tiled host-side to [128, ND, T] fp32 where
    # xt32[p, k, t] == x[t, k*128+p]. Router needs true-fp32 logits.
    xt32_d = nc.dram_tensor("xt32", [128, ND, T], f32, kind="ExternalInput").ap()
    # xb: row-major bf16 copy of x, gather source for expert buffers.
    xb_d = nc.dram_tensor("xb", [T, D], bf16, kind="ExternalInput").ap()
    wgt_d = nc.dram_tensor("wg_t", [D, E], f32, kind="ExternalInput").ap()
    w1_d = nc.dram_tensor("w1", [EXP_PER_CORE, D, MOE_I], bf16, kind="ExternalInput").ap()
    w3_d = nc.dram_tensor("w3", [EXP_PER_CORE, D, MOE_I], bf16, kind="ExternalInput").ap()
    w2_d = nc.dram_tensor("w2", [EXP_PER_CORE, MOE_I, D], bf16, kind="ExternalInput").ap()
    ws13_d = nc.dram_tensor("ws13", [D, 2 * SI_LOC], bf16, kind="ExternalInput").ap()
    ws2_d = nc.dram_tensor("ws2", [SI_LOC, D], bf16, kind="ExternalInput").ap()
    ident_d = nc.dram_tensor("ident128", [128, 128], f32, kind="ExternalInput").ap()
    tokcol_d = nc.dram_tensor("tokcol", [128, NT], f32, kind="ExternalInput").ap()
    out_d = nc.dram_tensor("out", [T, D], bf16, kind="ExternalOutput").ap()

    # NOTE on ordering: Tile's dependency tracking proved unreliable for
    # indirect-DMA writes followed by reads on a different DMA queue. All
    # writers/readers of table_dram and out_d therefore go through the SAME
    # queue (gpsimd / SWDGE qPoolDynamic): per-engine descriptor rings drain
    # in FIFO order, so program order implies completion order. Queue order:
    # table init -> table scatters (wait on DVE idx via semaphores) ->
    # e0 meta -> e0 gathers -> e1 meta -> shared-out writes -> e1 gathers ->
    # e0 scatter-adds -> e1 scatter-adds.
    with tile.TileContext(nc) as tc:
        with (
            tc.tile_pool(name="dram", bufs=1, space="DRAM") as dramp,
            tc.tile_pool(name="const", bufs=1) as constp,
            tc.tile_pool(name="xtres", bufs=1) as xtresp,
            tc.tile_pool(name="disp", bufs=1) as dispp,
            tc.tile_pool(name="ps_t", bufs=2, space="PSUM") as ps_t,
            tc.tile_pool(name="ps_mm", bufs=6, space="PSUM") as ps_mm,
            tc.tile_pool(name="small", bufs=2) as smallp,
        ):
            table_dram = dramp.tile([EXP_PER_CORE * CAP, 2], f32)  # (tok, w)

            # ---- constants ----
            identf = constp.tile([128, 128], f32)
            nc.sync.dma_start(out=identf[:], in_=ident_d)
            identb = constp.tile([128, 128], bf16)
            nc.vector.tensor_copy(identb[:], identf[:])
            tokcol = constp.tile([128, NT], f32)
            nc.sync.dma_start(out=tokcol[:], in_=tokcol_d)
            wgt_sb = constp.tile([128, ND, E], f32)
            nc.sync.dma_start(out=wgt_sb[:], in_=wgt_d.rearrange("(k p) e -> p k e", p=128))

            # init table to sentinel
            sent_sb = constp.tile([128, 2], f32)
            nc.vector.memset(sent_sb[:], SENTINEL)
            n_tab_tiles = EXP_PER_CORE * CAP // 128  # 10
            for t in range(n_tab_tiles):
                nc.gpsimd.dma_start(out=table_dram[t * 128:(t + 1) * 128, :],
                                    in_=sent_sb[:])

            # resident bf16 transposed activations [d-part, d-tile, tok]
            xT = xtresp.tile([128, ND, T], bf16)

            # dispatch-state tiles (live until the table scatters)
            wsel = dispp.tile([128, NT, E], f32)     # prob * mask
            maskT = dispp.tile([E, T], f32)
            posTx = dispp.tile([E, T], f32)          # exclusive prefix count
            pos = dispp.tile([128, NT, E], f32)

            # ============ Phase 0: load xT, cast to bf16, router ===========
            with (
                tc.tile_pool(name="xload", bufs=2) as xloadp,
                tc.tile_pool(name="router", bufs=1) as routerp,
            ):
                scT = routerp.tile([E, T], f32)            # raw logits transposed
                scores = routerp.tile([128, NT, E], f32)   # softmax probs
                QW = 256  # tokens per load chunk
                for q in range(T // QW):
                    xh = xloadp.tile([128, ND, QW], f32, tag="xh")
                    nc.sync.dma_start(out=xh[:], in_=xt32_d[:, :, q * QW:(q + 1) * QW])
                    # cast to resident bf16
                    for k in range(ND):
                        nc.vector.tensor_copy(xT[:, k, q * QW:(q + 1) * QW], xh[:, k, :])
                    # router logits, true fp32: scT = wg.T @ xT
                    pscs = []
                    for t2 in range(QW // 128):
                        pscs.append(ps_mm.tile([E, 128], f32, space="PSUM", tag="mm", name="psc"))
                    for k in range(ND):
                        for t2 in range(QW // 128):
                            nc.tensor.matmul(out=pscs[t2][:], lhsT=wgt_sb[:, k, :],
                                             rhs=xh[:, k, t2 * 128:(t2 + 1) * 128],
                                             start=(k == 0), stop=(k == ND - 1))
                    for t2 in range(QW // 128):
                        ti = q * (QW // 128) + t2
                        nc.vector.tensor_copy(scT[:, ti * 128:(ti + 1) * 128], pscs[t2][:])

                # ---- softmax + top-4 selection (two passes: keep the PE
                # free-running; the per-tile DVE chains pipeline behind the
                # score transposes instead of gating them one by one) ----
                logit = routerp.tile([128, NT, E], f32)
                mask = routerp.tile([128, NT, E], f32)   # top-4 one-hot
                for ti in range(NT):
                    pst = ps_t.tile([128, E], f32, space="PSUM", tag="pst")
                    nc.tensor.transpose(out=pst[:], in_=scT[:, ti * 128:(ti + 1) * 128],
                                        identity=identf[:E, :E])
                    nc.vector.tensor_copy(logit[:, ti, :], pst[:])
                for ti in range(NT):
                    expv = smallp.tile([128, E], f32, tag="expv")
                    ssum = smallp.tile([128, 1], f32, tag="ssum")
                    nc.scalar.activation(expv[:], logit[:, ti, :], AF.Exp, accum_out=ssum[:])
                    rsum = smallp.tile([128, 1], f32, tag="rsum")
                    nc.vector.reciprocal(rsum[:], ssum[:])
                    nc.vector.tensor_scalar_mul(scores[:, ti, :], expv[:], rsum[:, :1])
                    v8 = smallp.tile([128, 8], f32, tag="v8")
                    nc.vector.max(out=v8[:], in_=scores[:, ti, :])
                    nc.vector.tensor_scalar(
                        out=mask[:, ti, :], in0=scores[:, ti, :],
                        scalar1=v8[:, 3:4], scalar2=None, op0=ALU.is_ge)
                    nc.vector.tensor_tensor(
                        out=wsel[:, ti, :], in0=scores[:, ti, :], in1=mask[:, ti, :],
                        op=ALU.mult)
                for ti in range(NT):
                    pst2 = ps_t.tile([E, 128], f32, space="PSUM", tag="pst")
                    nc.tensor.transpose(out=pst2[:], in_=mask[:, ti, :], identity=identf[:])
                    nc.vector.tensor_copy(maskT[:, ti * 128:(ti + 1) * 128], pst2[:])

            # ---- positions via prefix scan along tokens (DVE) ----
            nc.vector.tensor_tensor_scan(
                out=posTx[:], data0=maskT[:], data1=maskT[:], initial=0.0,
                op0=ALU.add, op1=ALU.bypass)  # inclusive cumsum
            nc.vector.tensor_tensor(out=posTx[:], in0=posTx[:], in1=maskT[:],
                                    op=ALU.subtract)  # exclusive
            # fold top-4 selection into the positions: unselected -> ~1e6,
            # so the dispatch idx math needs no mask tile downstream.
            nc.vector.tensor_scalar(
                out=maskT[:], in0=maskT[:], scalar1=-1.0e6, scalar2=1.0e6,
                op0=ALU.mult, op1=ALU.add)
            nc.vector.tensor_tensor(out=posTx[:], in0=posTx[:], in1=maskT[:],
                                    op=ALU.add)

            # ============ Phase 1: shared expert + dispatch + routed =======
            with (
                tc.tile_pool(name="shw", bufs=1) as shwp,
                tc.tile_pool(name="shev", bufs=1) as shevp,
                tc.tile_pool(name="buf", bufs=1) as bufp,
                tc.tile_pool(name="gbuf", bufs=2) as gbufp,
                tc.tile_pool(name="wstream", bufs=4) as wstream,
                tc.tile_pool(name="odn", bufs=2) as odnp,
                tc.tile_pool(name="tw", bufs=1) as twp,
            ):
                # --- meta readback + token gathers ---
                tok_is = [[None] * NS for _ in range(EXP_PER_CORE)]
                w_cols = [[None] * NS for _ in range(EXP_PER_CORE)]
                gbufs = [[None] * NS for _ in range(EXP_PER_CORE)]

                def readback(j):
                    tok_i = twp.tile([128, NS], i32, tag=f"tok_{j}", name="tok_i")
                    w_col = twp.tile([128, NS], f32, tag=f"w_{j}", name="w_col")
                    for s in range(NS):
                        meta = smallp.tile([128, 2], f32, tag="meta")
                        nc.gpsimd.dma_start(
                            out=meta[:],
                            in_=table_dram[j * CAP + s * 128: j * CAP + (s + 1) * 128, :])
                        nc.vector.tensor_copy(tok_i[:, s:s + 1], meta[:, 0:1])
                        nc.vector.tensor_copy(w_col[:, s:s + 1], meta[:, 1:2])
                        tok_is[j][s] = tok_i[:, s:s + 1]
                        w_cols[j][s] = w_col[:, s:s + 1]

                def gather(j):
                    for s in range(NS):
                        gb = gbufp.tile([128, D], bf16, tag="gb", name="gb")
                        nc.vector.memset(gb[:], 0.0)
                        nc.gpsimd.indirect_dma_start(
                            out=gb[:], out_offset=None,
                            in_=xb_d,
                            in_offset=bass.IndirectOffsetOnAxis(
                                ap=tok_is[j][s], axis=0),
                            bounds_check=T - 1, oob_is_err=False)
                        gbufs[j][s] = gb

                def dispatch_and_gathers():
                    readback(0)
                    gather(0)
                    readback(1)

                # --- shared expert up: h = silu(ws1.T x) * (ws3.T x) ---
                h_sh = shwp.tile([128, 3, T], bf16)  # [i-part, m(128/128/96), tok]
                ws13_sb = shwp.tile([128, ND, 2 * SI_LOC], bf16)
                nc.sync.dma_start(out=ws13_sb[:],
                                  in_=ws13_d.rearrange("(k p) i -> p k i", p=128))
                m_sizes = [128, 128, SI_LOC - 256]  # 128,128,96

                def shared_up_half(half):
                    t0 = half * (T // 2)
                    gps = [[None] * 2 for _ in range(3)]
                    for m in range(3):
                        for n in range(2):
                            gps[m][n] = ps_mm.tile([128, 512], f32, space="PSUM",
                                                   tag="mm", name="shg")
                    for k in range(ND):
                        for m in range(3):
                            ms = m_sizes[m]
                            for n in range(2):
                                nc.tensor.matmul(
                                    out=gps[m][n][:ms, :],
                                    lhsT=ws13_sb[:, k, m * 128:m * 128 + ms],
                                    rhs=xT[:, k, t0 + n * 512:t0 + (n + 1) * 512],
                                    start=(k == 0), stop=(k == ND - 1))
                    for m in range(3):
                        ms = m_sizes[m]
                        for n in range(2):
                            nc.scalar.activation(
                                h_sh[:ms, m, t0 + n * 512:t0 + (n + 1) * 512],
                                gps[m][n][:ms, :], AF.Silu)
                    ups = [[None] * 2 for _ in range(3)]
                    for m in range(3):
                        for n in range(2):
                            ups[m][n] = ps_mm.tile([128, 512], f32, space="PSUM",
                                                   tag="mm", name="shu")
                    for k in range(ND):
                        for m in range(3):
                            ms = m_sizes[m]
                            for n in range(2):
                                nc.tensor.matmul(
                                    out=ups[m][n][:ms, :],
                                    lhsT=ws13_sb[:, k, SI_LOC + m * 128:SI_LOC + m * 128 + ms],
                                    rhs=xT[:, k, t0 + n * 512:t0 + (n + 1) * 512],
                                    start=(k == 0), stop=(k == ND - 1))
                    for m in range(3):
                        ms = m_sizes[m]
                        for n in range(2):
                            sl = h_sh[:ms, m, t0 + n * 512:t0 + (n + 1) * 512]
                            nc.vector.tensor_tensor(
                                out=sl, in0=sl,
                                in1=ups[m][n][:ms, :], op=ALU.mult)

                shared_up_half(0)

                # pos transposes sit between the shared halves on the PE
                # queue: the scan latency hides under half 0's matmuls.
                for ti in range(NT):
                    pst = ps_t.tile([128, E], f32, space="PSUM", tag="pst")
                    nc.tensor.transpose(out=pst[:], in_=posTx[:, ti * 128:(ti + 1) * 128],
                                        identity=identf[:E, :E])
                    nc.vector.tensor_copy(pos[:, ti, :], pst[:])

                # --- dispatch-table scatters: FIRST on the gpsimd queue
                # (after table init); they wait on the DVE idx/pay tiles via
                # semaphores, so everything queued behind them stays ordered.
                for j in range(EXP_PER_CORE):
                    # pos already encodes selection (+1e6 when unselected)
                    idx_f = dispp.tile([128, NT], f32, tag="idxf")
                    nc.vector.tensor_scalar_add(idx_f[:], pos[:, :, j],
                                                float(j * CAP))
                    idx_i = dispp.tile([128, NT], i32, tag="idxi")
                    nc.vector.tensor_copy(idx_i[:], idx_f[:])
                    pay = dispp.tile([128, NT, 2], f32, tag="pay")
                    nc.vector.tensor_copy(pay[:, :, 0], tokcol[:])
                    nc.vector.tensor_copy(pay[:, :, 1], wsel[:, :, j])
                    for ti in range(NT):
                        nc.gpsimd.indirect_dma_start(
                            out=table_dram[:],
                            out_offset=bass.IndirectOffsetOnAxis(
                                ap=idx_i[:, ti:ti + 1], axis=0),
                            in_=pay[:, ti, :],
                            in_offset=None,
                            bounds_check=(j + 1) * CAP - 1,
                            oob_is_err=False,
                        )


                dispatch_and_gathers()

                shared_up_half(1)

                # --- gathered-buffer transposes (PE, cheap) ---
                def transpose_gathered(j, bufT):
                    for s in range(NS):
                        for dj in range(ND):
                            pst = ps_t.tile([128, 128], bf16, space="PSUM", tag="pst")
                            nc.tensor.transpose(
                                out=pst[:],
                                in_=gbufs[j][s][:, dj * 128:(dj + 1) * 128],
                                identity=identb[:])
                            nc.vector.tensor_copy(bufT[:, dj, s * 128:(s + 1) * 128],
                                                  pst[:])

                # --- shared expert down + output init write ---
                ws2_sb = shwp.tile([128, 3, D], bf16)  # k-stripes of ws2 (96 pad)
                for m in range(3):
                    ms = m_sizes[m]
                    nc.sync.dma_start(out=ws2_sb[:ms, m, :], in_=ws2_d[m * 128:m * 128 + ms, :])
                for ms_i in range(NT):  # output token tiles
                    o_sh = shevp.tile([128, D], bf16, tag="o_sh", bufs=1, name="o_sh")
                    for n in range(4):
                        op = ps_mm.tile([128, 512], f32, space="PSUM", tag="mm", name="shd")
                        for k in range(3):
                            ks = m_sizes[k]
                            nc.tensor.matmul(
                                out=op[:], lhsT=h_sh[:ks, k, ms_i * 128:(ms_i + 1) * 128],
                                rhs=ws2_sb[:ks, k, n * 512:(n + 1) * 512],
                                start=(k == 0), stop=(k == 2))
                        nc.scalar.activation(o_sh[:, n * 512:(n + 1) * 512], op[:], AF.Copy)
                    nc.gpsimd.dma_start(out=out_d[ms_i * 128:(ms_i + 1) * 128, :], in_=o_sh[:])

                # expert-0 transposes AFTER shared-down: its 40us of PE work
                # covers the dispatch->gather latency on the gpsimd queue.
                bufT0 = bufp.tile([128, ND, CAP], bf16, tag="bufT", name="bufT")
                transpose_gathered(0, bufT0)

                # expert-1 gathers queue behind the shared-out writes so they
                # never head-of-line block them; they still land well before
                # expert 1's transposes need them.
                gather(1)

                # ============ routed experts ============
                NH = CAP // 2  # 320, psum half-width
                m_groups = [(0, 2), (2, 2), (4, 2), (6, 2), (8, 2), (10, 1)]
                for j in range(EXP_PER_CORE):
                    if j > 0:
                        bufT = bufp.tile([128, ND, CAP], bf16, tag="bufT", name="bufT")
                        transpose_gathered(j, bufT)
                    else:
                        bufT = bufT0

                    # --- up projections: hT[i, slot] ---
                    hT = bufp.tile([128, NI, CAP], bf16, tag="hT", name="hT")
                    for (m0, mcnt) in m_groups:
                        # gate pass (whole m-group weight block in one DMA)
                        wg1 = wstream.tile([128, ND, 256], bf16, tag="wgrp", bufs=2,
                                           name="wg1")
                        nc.sync.dma_start(
                            out=wg1[:, :, :mcnt * 128],
                            in_=w1_d[j, :, m0 * 128:(m0 + mcnt) * 128].rearrange(
                                "(k p) i -> p k i", p=128))
                        gps = []
                        for _ in range(mcnt * 2):
                            gps.append(ps_mm.tile([128, NH], f32, space="PSUM",
                                                  tag="mm", name="gps"))
                        for k in range(ND):
                            for mi in range(mcnt):
                                for nh in range(2):
                                    nc.tensor.matmul(
                                        out=gps[mi * 2 + nh][:],
                                        lhsT=wg1[:, k, mi * 128:(mi + 1) * 128],
                                        rhs=bufT[:, k, nh * NH:(nh + 1) * NH],
                                        start=(k == 0), stop=(k == ND - 1))
                        for mi in range(mcnt):
                            for nh in range(2):
                                nc.scalar.activation(
                                    hT[:, m0 + mi, nh * NH:(nh + 1) * NH],
                                    gps[mi * 2 + nh][:], AF.Silu)
                        # up pass
                        wg3 = wstream.tile([128, ND, 256], bf16, tag="wgrp", bufs=2,
                                           name="wg3")
                        nc.sync.dma_start(
                            out=wg3[:, :, :mcnt * 128],
                            in_=w3_d[j, :, m0 * 128:(m0 + mcnt) * 128].rearrange(
                                "(k p) i -> p k i", p=128))
                        ups = []
                        for _ in range(mcnt * 2):
                            ups.append(ps_mm.tile([128, NH], f32, space="PSUM",
                                                  tag="mm", name="ups"))
                        for k in range(ND):
                            for mi in range(mcnt):
                                for nh in range(2):
                                    nc.tensor.matmul(
                                        out=ups[mi * 2 + nh][:],
                                        lhsT=wg3[:, k, mi * 128:(mi + 1) * 128],
                                        rhs=bufT[:, k, nh * NH:(nh + 1) * NH],
                                        start=(k == 0), stop=(k == ND - 1))
                        for mi in range(mcnt):
                            for nh in range(2):
                                sl = hT[:, m0 + mi, nh * NH:(nh + 1) * NH]
                                nc.vector.tensor_tensor(
                                    out=sl, in0=sl,
                                    in1=ups[mi * 2 + nh][:], op=ALU.mult)

                    # --- down projection + weighted chunked scatter-add ---
                    o_dns = [None] * NS
                    for n in range(4):
                        dps = []
                        for s in range(NS):
                            dps.append(ps_mm.tile([128, 512], f32, space="PSUM",
                                                  tag="mm", name="dps"))
                        for k2 in range(6):  # 11 k-tiles as 5 pairs + 1
                            cnt = 2 if k2 < 5 else 1
                            wc = wstream.tile([128, 2, 512], bf16, tag="w2c", bufs=2,
                                              name="wc2")
                            nc.sync.dma_start(
                                out=wc[:, :cnt, :],
                                in_=w2_d[j, k2 * 256:k2 * 256 + cnt * 128,
                                         n * 512:(n + 1) * 512].rearrange(
                                    "(k p) d -> p k d", p=128))
                            for kk in range(cnt):
                                k = k2 * 2 + kk
                                for s in range(NS):
                                    nc.tensor.matmul(
                                        out=dps[s][:], lhsT=hT[:, k, s * 128:(s + 1) * 128],
                                        rhs=wc[:, kk, :], start=(k == 0), stop=(k == NI - 1))
                        for s in range(NS):
                            if n % 2 == 0:
                                o_dns[s] = odnp.tile([128, 1024], bf16, tag="o_dn",
                                                     bufs=5, name="o_dn")
                            nc.vector.tensor_scalar_mul(
                                o_dns[s][:, (n % 2) * 512:(n % 2 + 1) * 512],
                                dps[s][:], w_cols[j][s])
                            if n % 2 == 1:
                                nc.gpsimd.indirect_dma_start(
                                    out=out_d,
                                    out_offset=bass.IndirectOffsetOnAxis(
                                        ap=tok_is[j][s], axis=0),
                                    in_=o_dns[s][:],
                                    in_offset=None,
                                    element_offset=(n - 1) * 512,
                                    bounds_check=T - 1, oob_is_err=False,
                                    compute_op=ALU.add)

    nc.compile()
    return nc


def make_in_maps(inputs):
    """Build per-core input maps from the full (unsharded) inputs."""
    import ml_dtypes
    bf16 = ml_dtypes.bfloat16

    x = np.ascontiguousarray(np.asarray(inputs["hidden_states"], dtype=np.float32))
    w_gate = np.asarray(inputs["w_gate"], dtype=np.float32)
    w1 = np.asarray(inputs["w1"], dtype=np.float32)
    w3 = np.asarray(inputs["w3"], dtype=np.float32)
    w2 = np.asarray(inputs["w2"], dtype=np.float32)
    ws1 = np.asarray(inputs["ws1"], dtype=np.float32)
    ws3 = np.asarray(inputs["ws3"], dtype=np.float32)
    ws2 = np.asarray(inputs["ws2"], dtype=np.float32)

    # xt32[p, k, t] = x[t, k*128+p]
    xt32 = np.ascontiguousarray(x.T.reshape(ND, 128, T).transpose(1, 0, 2))
    xb = np.ascontiguousarray(x.astype(bf16))
    ident128 = np.eye(128, dtype=np.float32)
    tokcol = (np.arange(NT, dtype=np.float32)[None, :] * 128.0
              + np.arange(128, dtype=np.float32)[:, None])

    in_maps = []
    for c in range(NCORES):
        e0 = EXP_PER_CORE * c
        # permute router columns: local experts first, so the kernel's
        # dispatch math can address them at compile-time columns 0..1.
        perm = [e0, e0 + 1] + [e for e in range(E) if e not in (e0, e0 + 1)]
        wg_t = np.ascontiguousarray(w_gate.T[:, perm])  # [D, E]
        in_maps.append({
            "xt32": xt32,
            "xb": xb,
            "wg_t": wg_t,
            "w1": np.ascontiguousarray(w1[e0:e0 + EXP_PER_CORE].astype(bf16)),
            "w3": np.ascontiguousarray(w3[e0:e0 + EXP_PER_CORE].astype(bf16)),
            "w2": np.ascontiguousarray(w2[e0:e0 + EXP_PER_CORE].astype(bf16)),
            "ws13": np.ascontiguousarray(np.concatenate(
                [ws1[:, c * SI_LOC:(c + 1) * SI_LOC],
                 ws3[:, c * SI_LOC:(c + 1) * SI_LOC]], axis=1).astype(bf16)),
            "ws2": np.ascontiguousarray(ws2[c * SI_LOC:(c + 1) * SI_LOC, :].astype(bf16)),
            "ident128": ident128,
            "tokcol": np.ascontiguousarray(tokcol),
        })
    return in_maps


_NC_CACHE = None


def kernel(**inputs) -> np.ndarray:
    global _NC_CACHE
    from concourse.bass_utils import run_bass_kernel_spmd

    if _NC_CACHE is None:
        _NC_CACHE = build_nc()
    nc = _NC_CACHE
    in_maps = make_in_maps(inputs)
    res = run_bass_kernel_spmd(nc, in_maps, list(range(NCORES)))
    out = np.zeros((T, D), dtype=np.float32)
    for c in range(NCORES):
        out += res.results[c]["out"]
    return out
